# revision 1
# baseline (speedup 1.0000x reference)
"""Causal GQA self-attention (B=4, T=2048, C=2048, 16 heads / 4 kv-heads,
l2-normalized q,k) on 8 Trainium2 NeuronCores.

Sharding: core pair (2b, 2b+1) handles batch b. Within a pair, queries are
split stride-2 by row parity (parity p takes rows p::2), which balances the
causal work and keeps the compiled program identical on all 8 cores. All
per-core differences are data: odd-parity cores receive x with adjacent
rows pair-swapped, so the program's fixed `rows 0::2` query gather selects
the odd rows while every key row remains present; the swapped in-chunk key
order is absorbed into that core's causal-mask tiles (a kernel input).

Per core:
  xT      = PE-transpose(x_b) in f32r (full-rate fp32 matmul format)
  kT, vT  = Wk/Wv projections of full T (f32r), k l2-normalized via
            ones-matrix matmul partition reduction (result arrives
            partition-broadcast), v re-transposed on PE
  qT      = Wq projection of this core's 1024 query rows, computed from a
            compact strided extraction xqT so Wq is DMA'd only once
  attn    = scores^T layout sT[k,q] (bf16), exp on ACT with fused 1/sqrt(hd)
            scale (no max-subtraction needed: l2norm bounds |s| <= 1),
            causal masking = bf16 0/1 multiply, denominators via all-ones
            matmul, AV with v as lhsT, yT normalized by fast reciprocal
  out     = Wproj^T consumes yT directly (bf16), written feature-major
Host transposes the [C, 1024] result and scatters rows p::2 of batch b.
"""

import numpy as np
import ml_dtypes

import concourse.bacc as bacc
import concourse.mybir as mybir
import concourse.tile as tile
from concourse.bass_utils import run_bass_kernel_spmd

B, T, C = 4, 2048, 2048
NH, NKV, HD = 16, 4, 128
KV = 512            # k (and v) projection width
P = 128
SCALE = 1.0 / float(np.sqrt(HD))
N_CORES = 8

F32 = mybir.dt.float32
F32R = mybir.dt.float32r
BF16 = mybir.dt.bfloat16
FP16 = mybir.dt.float16
Exp = mybir.ActivationFunctionType.Exp
Sqrt = mybir.ActivationFunctionType.Sqrt
MUL = mybir.AluOpType.mult

NST = 4             # four 512-row T strips
STS = 4             # 128-row blocks per strip
NCC = 16            # C contraction chunks of 128
NM_K = KV // P      # 4  (kdim / vdim output chunks)
NM_Q = C // P       # 16 (qdim output chunks)
LQ = T // 2         # 1024 local query rows per core
SS_W = 512          # local queries per attention strip


def build():
    nc = bacc.Bacc("TRN2", target_bir_lowering=False, debug=False,
                   num_devices=N_CORES)
    xb = nc.declare_dram_parameter("xb", [T, C], F32R, isOutput=False)
    wq = nc.declare_dram_parameter("wq", [C, C], F32R, isOutput=False)
    wk = nc.declare_dram_parameter("wk", [C, KV], F32R, isOutput=False)
    wv = nc.declare_dram_parameter("wv", [C, KV], F32R, isOutput=False)
    wp = nc.declare_dram_parameter("wp", [C, C], BF16, isOutput=False)
    masks = nc.declare_dram_parameter("masks", [8, P, SS_W], FP16,
                                      isOutput=False)
    ident_in = nc.declare_dram_parameter("ident", [P, P], F32R, isOutput=False)
    ones_in = nc.declare_dram_parameter("onesb", [P, P], BF16, isOutput=False)
    onesh_in = nc.declare_dram_parameter("onesh", [P, P], FP16, isOutput=False)
    out = nc.declare_dram_parameter("out", [C, LQ], F32, isOutput=True)

    with tile.TileContext(nc) as tc:
        with (
            tc.tile_pool(name="cst", bufs=1) as cst,
            tc.tile_pool(name="qT", bufs=1) as p_qT,
            tc.tile_pool(name="kT", bufs=1) as p_kT,
            tc.tile_pool(name="vsb", bufs=1) as p_v,
        ):
            ident = cst.tile([P, P], F32R)
            nc.sync.dma_start(ident[:], ident_in[:])
            ones_bf = cst.tile([P, P], BF16)
            nc.sync.dma_start(ones_bf[:], ones_in[:])
            ones_h = cst.tile([P, P], FP16)
            nc.sync.dma_start(ones_h[:], onesh_in[:])

            qT_sb = p_qT.tile([P, NM_Q, LQ], BF16)       # 32 KB/part
            kT_sb = p_kT.tile([P, NM_K, T], BF16)        # 16 KB/part
            v_sb = p_v.tile([P, NCC, KV], FP16)          # 16 KB/part

            # ------------- Phases K (kv proj) and Q (q proj) -------------
            # query-row extraction bounced through DRAM scratch:
            # xqs[:, cc, 64*(4*st+ts) + i] = x[t0+ts*128 + 2i, cc*128+p].T
            xqs = nc.dram_tensor("xqs", [P, NCC, 16 * 64], F32R)
            if True:
                with (
                    tc.tile_pool(name="xT", bufs=1) as p_xT,
                    tc.tile_pool(name="vT", bufs=1) as p_vT,
                    tc.tile_pool(name="wt", bufs=4) as p_wt,
                    tc.tile_pool(name="xin", bufs=4) as p_xin,
                    tc.tile_pool(name="nrm", bufs=2) as p_nrm,
                    tc.tile_pool(name="sq", bufs=2) as p_sq,
                    tc.tile_pool(name="stg", bufs=1) as p_stg,
                    tc.tile_pool(name="pt", bufs=3, space="PSUM") as ps_t,
                    tc.tile_pool(name="acc", bufs=3, space="PSUM") as ps_a,
                    tc.tile_pool(name="pssq", bufs=2, space="PSUM") as ps_q,
                ):
                    for st in range(NST):
                        t0 = st * 512
                        tsl = slice(t0, t0 + 512)
                        # xT[:, ts, cc, :] = x[t0+ts*128.., cc*128..].T
                        xT = p_xT.tile([P, STS, NCC, P], F32R, tag="xT")
                        for ts in range(STS):
                            x_sb = p_xin.tile([P, C], F32R, tag="xin")
                            nc.sync.dma_start(
                                x_sb[:],
                                xb[t0 + ts * P: t0 + (ts + 1) * P, :])
                            for c4 in range(4):
                                pt = ps_t.tile([P, 4, P], F32R, tag="pt")
                                for j in range(4):
                                    cc = c4 * 4 + j
                                    nc.tensor.transpose(
                                        pt[:, j, :],
                                        x_sb[:, cc * P:(cc + 1) * P],
                                        ident[:])
                                dst = xT[:, ts, c4 * 4:(c4 + 1) * 4, :]
                                if c4 % 2 == 0:
                                    nc.scalar.copy(out=dst, in_=pt[:])
                                else:
                                    nc.vector.tensor_copy(dst, pt[:])
                        # compact strided query-row extraction -> DRAM
                        stg = p_stg.tile([P, NCC, STS, 64], F32R, tag="stg")
                        for c2 in range(4):
                            for cc in range(c2 * 4, c2 * 4 + 4):
                                nc.vector.tensor_copy(stg[:, cc],
                                                      xT[:, :, cc, 0::2])
                            nc.sync.dma_start(
                                xqs[:, c2 * 4:(c2 + 1) * 4,
                                    st * 256:(st + 1) * 256],
                                stg[:, c2 * 4:(c2 + 1) * 4])

                        # ---- kT projection + l2norm(k) ----
                        ssqk = ps_q.tile([P, 512], F32, tag="ssq")
                        for m in range(NM_K):
                            wk_t = p_wt.tile([P, NCC, P], F32R, tag="wt")
                            nc.sync.dma_start(
                                wk_t[:],
                                wk[:, m * P:(m + 1) * P].rearrange(
                                    "(cc p) f -> p cc f", p=P))
                            pk = ps_a.tile([P, 512], F32, tag="acc")
                            for cc in range(NCC):
                                nc.tensor.matmul(
                                    pk[:], wk_t[:, cc, :], xT[:, :, cc, :],
                                    start=(cc == 0), stop=(cc == NCC - 1))
                            nc.scalar.copy(out=kT_sb[:, m, tsl], in_=pk[:])
                            sq = p_sq.tile([P, 512], BF16, tag="sq")
                            nc.vector.tensor_tensor(
                                sq[:], kT_sb[:, m, tsl], kT_sb[:, m, tsl],
                                MUL)
                            nc.tensor.matmul(
                                ssqk[:], ones_bf[:], sq[:],
                                start=(m == 0), stop=(m == NM_K - 1))
                        nrm = p_nrm.tile([P, 512], F32, tag="nrm")
                        nc.scalar.activation(nrm[:], ssqk[:], Sqrt)
                        rk = p_nrm.tile([P, 512], F32, tag="rk")
                        nc.vector.reciprocal_approx_fast(rk[:], nrm[:])
                        rk16 = p_sq.tile([P, 512], BF16, tag="rk16")
                        nc.vector.tensor_copy(rk16[:], rk[:])
                        nc.vector.tensor_tensor(
                            kT_sb[:, :, tsl], kT_sb[:, :, tsl],
                            rk16[:, None, :].to_broadcast([P, NM_K, 512]),
                            MUL)

                        # ---- vT projection, then transpose to v ----
                        vT = p_vT.tile([P, NM_K, 512], F32R, tag="vT")
                        for m in range(NM_K):
                            wv_t = p_wt.tile([P, NCC, P], F32R, tag="wt")
                            nc.sync.dma_start(
                                wv_t[:],
                                wv[:, m * P:(m + 1) * P].rearrange(
                                    "(cc p) f -> p cc f", p=P))
                            pv = ps_a.tile([P, 512], F32, tag="acc")
                            for cc in range(NCC):
                                nc.tensor.matmul(
                                    pv[:], wv_t[:, cc, :], xT[:, :, cc, :],
                                    start=(cc == 0), stop=(cc == NCC - 1))
                            nc.scalar.copy(out=vT[:, m, :], in_=pv[:])
                        for m in range(NM_K):
                            pt = ps_t.tile([P, 4, P], F32R, tag="pt")
                            for j in range(4):
                                nc.tensor.transpose(
                                    pt[:, j, :], vT[:, m, j * P:(j + 1) * P],
                                    ident[:])
                            for j in range(4):
                                nc.vector.tensor_copy(
                                    v_sb[:, st * 4 + j, m * P:(m + 1) * P],
                                    pt[:, j, :])

                # ---- Phase Q: q projection + l2norm over all 16 chunks ----
                with (
                    tc.tile_pool(name="xqh", bufs=1) as p_xqh,
                    tc.tile_pool(name="wtq", bufs=4) as p_wtq,
                    tc.tile_pool(name="nrmq", bufs=2) as p_nrmq,
                    tc.tile_pool(name="sqq", bufs=2) as p_sqq,
                    tc.tile_pool(name="accq", bufs=3, space="PSUM") as ps_aq,
                    tc.tile_pool(name="ssqq", bufs=2, space="PSUM") as ps_qq,
                ):
                    ssq = [ps_qq.tile([P, 512], F32, tag="ssq",
                                      name=f"ssq{_i}")
                           for _i in range(2)]
                    xqh = [p_xqh.tile([P, NCC, 512], F32R, tag=f"xqh{_i}",
                                      name=f"xqh{_i}")
                           for _i in range(2)]
                    for sp in range(2):
                        nc.sync.dma_start(
                            xqh[sp][:],
                            xqs[:, :, sp * 512:(sp + 1) * 512])
                    for m in range(NM_Q):
                        wq_t = p_wtq.tile([P, NCC, P], F32R, tag="wt")
                        nc.sync.dma_start(
                            wq_t[:],
                            wq[:, m * P:(m + 1) * P].rearrange(
                                "(cc p) f -> p cc f", p=P))
                        for sp in range(2):
                            qsl = slice(sp * 512, (sp + 1) * 512)
                            pq = ps_aq.tile([P, 512], F32, tag="acc")
                            for cc in range(NCC):
                                nc.tensor.matmul(
                                    pq[:], wq_t[:, cc, :],
                                    xqh[sp][:, cc, :],
                                    start=(cc == 0), stop=(cc == NCC - 1))
                            nc.scalar.copy(out=qT_sb[:, m, qsl], in_=pq[:])
                            sq = p_sqq.tile([P, 512], BF16, tag="sq")
                            nc.vector.tensor_tensor(
                                sq[:], qT_sb[:, m, qsl], qT_sb[:, m, qsl],
                                MUL)
                            nc.tensor.matmul(
                                ssq[sp][:], ones_bf[:], sq[:],
                                start=(m == 0), stop=(m == NM_Q - 1))
                    for sp in range(2):
                        qsl = slice(sp * 512, (sp + 1) * 512)
                        nrm = p_nrmq.tile([P, 512], F32, tag="nrm")
                        nc.scalar.activation(nrm[:], ssq[sp][:], Sqrt)
                        rq = p_nrmq.tile([P, 512], F32, tag="rq")
                        nc.vector.reciprocal_approx_fast(rq[:], nrm[:])
                        rq16 = p_sqq.tile([P, 512], BF16, tag="rq16")
                        nc.vector.tensor_copy(rq16[:], rq[:])
                        nc.vector.tensor_tensor(
                            qT_sb[:, :, qsl], qT_sb[:, :, qsl],
                            rq16[:, None, :].to_broadcast([P, NM_Q, 512]),
                            MUL)

            # -------- Phase A (attention), then Phase O (out-proj) --------
            with (
                tc.tile_pool(name="e", bufs=6) as p_e,
                tc.tile_pool(name="rd", bufs=3) as p_rd,
                tc.tile_pool(name="osb", bufs=2) as p_o,
                tc.tile_pool(name="wpt", bufs=2) as p_wp,
                tc.tile_pool(name="acce", bufs=3) as p_acc,
                tc.tile_pool(name="ps_s", bufs=2, space="PSUM") as ps_s,
                tc.tile_pool(name="ps_y", bufs=3, space="PSUM") as ps_y,
                tc.tile_pool(name="ps_d", bufs=1, space="PSUM") as ps_d,
                tc.tile_pool(name="msk", bufs=1) as p_msk,
                tc.tile_pool(name="yT", bufs=2) as p_yT,
            ):
                mask_sb = p_msk.tile([P, 8, SS_W], FP16)
                nc.sync.dma_start(mask_sb[:],
                                  masks.rearrange("j p f -> p j f"))
                for ss in range(2):
                    nk = 8 * (ss + 1)
                    yT = p_yT.tile([P, NH, SS_W], BF16, tag="yT")
                    lsl = slice(ss * SS_W, (ss + 1) * SS_W)
                    for h in range(NH):
                        g = h // 4
                        py = ps_y.tile([P, SS_W], F32, tag="y")
                        acc = p_acc.tile([P, 2, SS_W], FP16, tag="acc")
                        for kc2 in range(nk // 2):
                            psc = ps_s.tile([P, 2, SS_W], F32, tag="s")
                            for i in range(2):
                                kc = 2 * kc2 + i
                                nc.tensor.matmul(
                                    psc[:, i, :],
                                    kT_sb[:, g, kc * P:(kc + 1) * P],
                                    qT_sb[:, h, lsl],
                                    start=True, stop=True)
                            e = p_e.tile([P, 2, SS_W], FP16, tag="e")
                            nc.scalar.activation(e[:], psc[:], Exp,
                                                 scale=SCALE)
                            j0 = 2 * kc2 - 8 * ss
                            if j0 >= 0:  # partial (diagonal-band) pair
                                nc.vector.tensor_tensor(
                                    e[:], e[:], mask_sb[:, j0:j0 + 2, :],
                                    MUL)
                            # fp16 running sum of exp chunks (for the
                            # denominators; fp16 keeps the sum accurate);
                            # acc keeps two parallel partial sums, folded
                            # once at the end
                            if kc2 == 0:
                                nc.vector.tensor_copy(acc[:], e[:])
                            else:
                                nc.vector.tensor_tensor(
                                    acc[:], acc[:], e[:],
                                    mybir.AluOpType.add)
                            for i in range(2):
                                kc = 2 * kc2 + i
                                nc.tensor.matmul(
                                    py[:],
                                    v_sb[:, kc, g * P:(g + 1) * P],
                                    e[:, i, :],
                                    start=(kc == 0), stop=(kc == nk - 1))
                        accf = p_acc.tile([P, SS_W], FP16, tag="accf")
                        nc.vector.tensor_tensor(
                            accf[:], acc[:, 0, :], acc[:, 1, :],
                            mybir.AluOpType.add)
                        pden = ps_d.tile([P, SS_W], F32, tag="d")
                        nc.tensor.matmul(pden[:], ones_h[:], accf[:],
                                         start=True, stop=True)
                        rden = p_rd.tile([P, SS_W], F32, tag="rd")
                        nc.vector.reciprocal_approx_fast(rden[:], pden[:])
                        nc.vector.tensor_tensor(yT[:, h, :], py[:], rden[:],
                                                MUL)

                    # out-projection for this strip, right after its
                    # attention so it can fill PE gaps of the next phase
                    for og in range(NM_Q // 4):
                        wp_t = p_wp.tile([P, NH, 4 * P], BF16, tag="wpt")
                        nc.sync.dma_start(
                            wp_t[:],
                            wp[:, og * 4 * P:(og + 1) * 4 * P].rearrange(
                                "(hh p) f -> p hh f", p=P))
                        o_sb = p_o.tile([P, 4, SS_W], F32, tag="o")
                        for j in range(4):
                            po = ps_s.tile([P, SS_W], F32, tag="s")
                            for hh in range(NH):
                                nc.tensor.matmul(
                                    po[:],
                                    wp_t[:, hh, j * P:(j + 1) * P],
                                    yT[:, hh, :],
                                    start=(hh == 0), stop=(hh == NH - 1))
                            nc.vector.tensor_copy(o_sb[:, j, :], po[:])
                        nc.sync.dma_start(
                            out.rearrange("(og j p) q -> p og j q", p=P,
                                          j=4)[:, og, :, lsl],
                            o_sb[:])

    nc.compile()
    return nc


_NC = None


def _get_nc():
    global _NC
    if _NC is None:
        _NC = build()
    return _NC


def _make_masks(p: int) -> np.ndarray:
    j = np.arange(8)[:, None, None]
    k = np.arange(P)[None, :, None]
    q = np.arange(SS_W)[None, None, :]
    if p == 0:
        valid = (2 * q) >= (128 * j + k)
    else:
        # odd cores see pair-swapped rows: key at in-chunk position k is
        # global row 128*kc + (k ^ 1); queries are odd rows 2q+1
        valid = (2 * q + 1) >= (128 * j + (k ^ 1))
    return valid.astype(np.float16)


def kernel(x, Wq, Wkv, Wproj):
    x = np.asarray(x, dtype=np.float32)
    Wq = np.asarray(Wq, dtype=np.float32)
    Wkv = np.asarray(Wkv, dtype=np.float32)
    Wproj = np.asarray(Wproj, dtype=np.float32)

    wk = np.ascontiguousarray(Wkv[:, :KV])
    wv = np.ascontiguousarray(Wkv[:, KV:])
    wp16 = Wproj.astype(ml_dtypes.bfloat16)
    ident = np.eye(P, dtype=np.float32)
    onesb = np.ones((P, P), dtype=ml_dtypes.bfloat16)
    onesh = np.ones((P, P), dtype=np.float16)
    masks_by_p = [_make_masks(0), _make_masks(1)]

    in_maps = []
    for c in range(N_CORES):
        b, p = c // 2, c % 2
        if p == 0:
            xb_c = np.ascontiguousarray(x[b])
        else:
            # pair-swap rows (2i <-> 2i+1): the program's fixed "rows 0::2"
            # query gather then selects the odd rows, every key row is still
            # present, and the swapped in-chunk key order is absorbed into
            # this core's mask data.
            xb_c = np.ascontiguousarray(
                x[b].reshape(T // 2, 2, C)[:, ::-1, :].reshape(T, C))
        in_maps.append({
            "xb": xb_c,
            "wq": Wq, "wk": wk, "wv": wv, "wp": wp16,
            "masks": masks_by_p[p],
            "ident": ident, "onesb": onesb, "onesh": onesh,
        })

    nc = _get_nc()
    res = run_bass_kernel_spmd(nc, in_maps, list(range(N_CORES)),
                               trace=False)

    result = np.empty((B, T, C), dtype=np.float32)
    for c in range(N_CORES):
        b, p = c // 2, c % 2
        result[b, p::2, :] = res.results[c]["out"].T
    return result



# revision 11
# speedup vs baseline: 1.5474x; 1.5474x over previous
"""Causal GQA self-attention (B=4, T=2048, C=2048, 16 heads / 4 kv-heads,
l2-normalized q,k) on 8 Trainium2 NeuronCores.

Sharding: core pair (2b, 2b+1) handles batch b; parity p takes query rows
p::2 (odd cores receive x with adjacent rows pair-swapped so the fixed
"rows 0::2" gather selects odd rows; the swapped in-chunk key order is
absorbed into mask data).

Because q and k are l2-normalized, |scores| <= 1/sqrt(128), so
exp(s) = 1 + s + O(5e-3) and softmax is near-uniform.  The kernel exploits
this:
  - q/k/v projections run as fp8e4 DoubleRow matmuls (weights pre-scaled
    x32 on the host; the scale cancels in l2norm, and for v it is folded
    into Wproj/32).
  - attention over "full" (strictly-causal) key chunks is linearized:
    sum_k (1+s) v = sum_k v + c * q8^T (K^T V), with K^T V and sum_k k
    prefix states recomputed per 256-query strip (chunked linear
    attention).  The quadratic remainder is O(s^2/2) <= 4e-3 relative on
    near-uniform weights.
  - only the 4 diagonal key chunks per strip get exact exp (fp16) +
    causal-mask multiply + fp16 AV, keeping full accuracy where the
    softmax is sharp (short prefixes).
  - denominators: matmul column-sums (ones weights) of e16 plus the
    linearized ksum^T q8 term; +N_full added on DVE before reciprocal.
"""

from contextlib import ExitStack
from types import SimpleNamespace

import numpy as np
import ml_dtypes

import concourse.bacc as bacc
import concourse.mybir as mybir
import concourse.tile as tile
from concourse.bass_utils import run_bass_kernel_spmd

B, T, C = 4, 2048, 2048
NH, NKV, HD = 16, 4, 128
KV = 512
P = 128
N_CORES = 8

F32 = mybir.dt.float32
BF16 = mybir.dt.bfloat16
FP16 = mybir.dt.float16
FP8 = mybir.dt.float8e4
Exp = mybir.ActivationFunctionType.Exp
Sqrt = mybir.ActivationFunctionType.Sqrt
Copy = mybir.ActivationFunctionType.Copy
MUL = mybir.AluOpType.mult
DR = mybir.MatmulPerfMode.DoubleRow

NST = 4              # 512-token projection strips
NCC = 16             # C contraction chunks of 128
NM_K = KV // P       # 4
NM_Q = C // P        # 16
LQ = T // 2          # 1024 local query rows
NSA = 4              # attention strips
SW = LQ // NSA       # 256 local queries per attention strip

WS = 32.0                                    # host weight pre-scale
C0 = float(1.0 / (1024.0 * np.sqrt(128.0)))  # exp scale / KV16 / ksum scale

# experimental: DoubleRow with 16-bit operands (cost-model 2x; HW-legality
# unverified) — keep False unless the HW run confirms numerics
DR16_OUT = False
DR16_DIAG = False


def _phase_k(nc, tc, g):
    """x transpose, k/v projections + l2norm(k), per 512-token strip."""
    with ExitStack() as ctx:
        p_xT = ctx.enter_context(tc.tile_pool(name="xT", bufs=1))
        p_xin = ctx.enter_context(tc.tile_pool(name="xin", bufs=4))
        p_vT = ctx.enter_context(tc.tile_pool(name="vT", bufs=1))
        p_wt = ctx.enter_context(tc.tile_pool(name="wt", bufs=4))
        p_nrm = ctx.enter_context(tc.tile_pool(name="nrm", bufs=2))
        p_sq = ctx.enter_context(tc.tile_pool(name="sq", bufs=2))
        p_stg = ctx.enter_context(tc.tile_pool(name="stg", bufs=2))
        ps_t = ctx.enter_context(tc.tile_pool(name="pt", bufs=3,
                                              space="PSUM"))
        ps_a = ctx.enter_context(tc.tile_pool(name="acc", bufs=3,
                                              space="PSUM"))
        ps_q = ctx.enter_context(tc.tile_pool(name="pssq", bufs=2,
                                              space="PSUM"))
        for st in range(NST):
            _strip_k(nc, g, st, p_xT, p_xin, p_vT, p_wt, p_nrm, p_sq, p_stg,
                     ps_t, ps_a, ps_q)


def _strip_k(nc, g, st, p_xT, p_xin, p_vT, p_wt, p_nrm, p_sq, p_stg,
             ps_t, ps_a, ps_q):
    t0 = st * 512
    tsl = slice(t0, t0 + 512)
    # xT[:, cc, ts, :] = x[t0+ts*128.., cc*128..].T  (fp8)
    xT = p_xT.tile([P, NCC, NST, P], FP8, tag="xT")
    for ts in range(4):
        x_sb = p_xin.tile([P, C], BF16, tag="xin")
        nc.sync.dma_start(x_sb[:], g.xb[t0 + ts * P: t0 + (ts + 1) * P, :])
        for c4 in range(4):
            pt = ps_t.tile([P, 4, P], BF16, tag="pt")
            for j in range(4):
                cc = c4 * 4 + j
                nc.tensor.transpose(pt[:, j, :],
                                    x_sb[:, cc * P:(cc + 1) * P], g.ident[:])
            nc.scalar.copy(out=xT[:, c4 * 4:(c4 + 1) * 4, ts, :], in_=pt[:])
    # strided query-row extraction -> xqh (SBUF->SBUF DMA)
    stg = p_stg.tile([P, NCC, NST, 64], FP8, tag="stg")
    for cc in range(NCC):
        nc.gpsimd.tensor_copy(stg[:, cc], xT[:, cc, :, 0::2])
    nc.sync.dma_start(g.xqh[:, :, st * 256:(st + 1) * 256], stg[:])

    # ---- k projection (fp8 DR) + l2norm ----
    ssqk = ps_q.tile([P, 512], F32, tag="ssq")
    for m in range(NM_K):
        wk_t = p_wt.tile([P, NCC, P], FP8, tag="wt")
        nc.sync.dma_start(
            wk_t[:],
            g.wk[:, m * P:(m + 1) * P].rearrange("(cc p) f -> p cc f", p=P))
        pk = ps_a.tile([P, 512], F32, tag="acc")
        for c in range(8):
            nc.tensor.matmul(pk[:], wk_t[:, 2 * c:2 * c + 2, :],
                             xT[:, 2 * c:2 * c + 2, :, :],
                             start=(c == 0), stop=(c == 7), perf_mode=DR)
        nc.scalar.copy(out=g.kT_sb[:, m, tsl], in_=pk[:])
        sq = p_sq.tile([P, 512], BF16, tag="sq")
        nc.vector.tensor_tensor(sq[:], g.kT_sb[:, m, tsl], g.kT_sb[:, m, tsl],
                                MUL)
        nc.tensor.matmul(ssqk[:], g.ones_bf[:], sq[:],
                         start=(m == 0), stop=(m == NM_K - 1))
    nrm = p_nrm.tile([P, 512], F32, tag="nrm")
    nc.scalar.activation(nrm[:], ssqk[:], Sqrt, scale=float(2.0 ** -10))
    rk = p_nrm.tile([P, 512], F32, tag="rk")
    nc.vector.reciprocal_approx_fast(rk[:], nrm[:])
    rk16 = p_sq.tile([P, 512], BF16, tag="rk16")
    nc.vector.tensor_copy(rk16[:], rk[:])
    nc.vector.tensor_tensor(g.kT_sb[:, :, tsl], g.kT_sb[:, :, tsl],
                            rk16[:, None, :].to_broadcast([P, NM_K, 512]),
                            MUL)
    # fp8 copy + token-major transposes of normalized k
    nc.vector.tensor_copy(g.k8[:, :, tsl], g.kT_sb[:, :, tsl])
    for m in range(NM_K):
        ptk = ps_t.tile([P, 4, P], BF16, tag="pt")
        for j in range(4):
            nc.tensor.transpose(
                ptk[:, j, :], g.kT_sb[:, m, t0 + j * P: t0 + (j + 1) * P],
                g.ident[:])
        nc.vector.tensor_copy(g.kTok[:, st * 4:(st + 1) * 4, m, :], ptk[:])

    # ---- v projection (fp8 DR), transpose to token-major ----
    vT = p_vT.tile([P, NM_K, 512], BF16, tag="vT")
    for m in range(NM_K):
        wv_t = p_wt.tile([P, NCC, P], FP8, tag="wt")
        nc.sync.dma_start(
            wv_t[:],
            g.wv[:, m * P:(m + 1) * P].rearrange("(cc p) f -> p cc f", p=P))
        pv = ps_a.tile([P, 512], F32, tag="acc")
        for c in range(8):
            nc.tensor.matmul(pv[:], wv_t[:, 2 * c:2 * c + 2, :],
                             xT[:, 2 * c:2 * c + 2, :, :],
                             start=(c == 0), stop=(c == 7), perf_mode=DR)
        nc.scalar.copy(out=vT[:, m, :], in_=pv[:])
    for m in range(NM_K):
        ptv = ps_t.tile([P, 4, P], BF16, tag="pt")
        for j in range(4):
            nc.tensor.transpose(ptv[:, j, :], vT[:, m, j * P:(j + 1) * P],
                                g.ident[:])
        nc.vector.tensor_copy(
            g.v16[:, st * 4:(st + 1) * 4, m * P:(m + 1) * P], ptv[:])
        nc.vector.tensor_copy(
            g.v8[:, st * 4:(st + 1) * 4, m * P:(m + 1) * P], ptv[:])


def _phase_q_body(nc, tc, g):
    """q projection (fp8 DR) + l2norm over all 16 chunks."""
    with ExitStack() as ctx:
        p_qT = ctx.enter_context(tc.tile_pool(name="qTb", bufs=1))
        p_wtq = ctx.enter_context(tc.tile_pool(name="wtq", bufs=4))
        p_nrmq = ctx.enter_context(tc.tile_pool(name="nrmq", bufs=2))
        p_sqq = ctx.enter_context(tc.tile_pool(name="sqq", bufs=2))
        ps_aq = ctx.enter_context(tc.tile_pool(name="accq", bufs=3,
                                               space="PSUM"))
        ps_qq = ctx.enter_context(tc.tile_pool(name="ssqq", bufs=2,
                                               space="PSUM"))
        qT_sb = p_qT.tile([P, NM_Q, LQ], BF16)
        ssq = [ps_qq.tile([P, 512], F32, tag="ssq", name=f"ssq{i}")
               for i in range(2)]
        for m in range(NM_Q):
            wq_t = p_wtq.tile([P, NCC, P], FP8, tag="wt")
            nc.sync.dma_start(
                wq_t[:],
                g.wq[:, m * P:(m + 1) * P].rearrange("(cc p) f -> p cc f",
                                                     p=P))
            for sp in range(2):
                qsl = slice(sp * 512, (sp + 1) * 512)
                pq = ps_aq.tile([P, 512], F32, tag="acc")
                for c in range(8):
                    nc.tensor.matmul(pq[:], wq_t[:, 2 * c:2 * c + 2, :],
                                     g.xqh[:, 2 * c:2 * c + 2, qsl],
                                     start=(c == 0), stop=(c == 7),
                                     perf_mode=DR)
                nc.scalar.copy(out=qT_sb[:, m, qsl], in_=pq[:])
                sq = p_sqq.tile([P, 512], BF16, tag="sq")
                nc.vector.tensor_tensor(sq[:], qT_sb[:, m, qsl],
                                        qT_sb[:, m, qsl], MUL)
                nc.tensor.matmul(ssq[sp][:], g.ones_bf[:], sq[:],
                                 start=(m == 0), stop=(m == NM_Q - 1))
        for sp in range(2):
            qsl = slice(sp * 512, (sp + 1) * 512)
            nrm = p_nrmq.tile([P, 512], F32, tag="nrm")
            nc.scalar.activation(nrm[:], ssq[sp][:], Sqrt,
                                 scale=float(2.0 ** -10))
            rq = p_nrmq.tile([P, 512], F32, tag="rq")
            nc.vector.reciprocal_approx_fast(rq[:], nrm[:])
            rq16 = p_sqq.tile([P, 512], BF16, tag="rq16")
            nc.vector.tensor_copy(rq16[:], rq[:])
            nc.vector.tensor_tensor(qT_sb[:, :, qsl], qT_sb[:, :, qsl],
                                    rq16[:, None, :].to_broadcast(
                                        [P, NM_Q, 512]), MUL)
            nc.vector.tensor_copy(g.q8[:, :, qsl], qT_sb[:, :, qsl])


def _phase_a(nc, tc, g):
    """attention (linear full chunks + exact diagonal) and out-projection."""
    with ExitStack() as ctx:
        p_e = ctx.enter_context(tc.tile_pool(name="e", bufs=4))
        p_kvs = ctx.enter_context(tc.tile_pool(name="kvs", bufs=2))
        p_rd = ctx.enter_context(tc.tile_pool(name="rd", bufs=3))
        p_yT = ctx.enter_context(tc.tile_pool(name="yT", bufs=2))
        p_wp = ctx.enter_context(tc.tile_pool(name="wpt", bufs=2))
        p_o = ctx.enter_context(tc.tile_pool(name="osb", bufs=2))
        ps_s = ctx.enter_context(tc.tile_pool(name="ps_s", bufs=2,
                                              space="PSUM"))
        ps_yd = ctx.enter_context(tc.tile_pool(name="ps_yd", bufs=2,
                                               space="PSUM"))
        ps_kv = ctx.enter_context(tc.tile_pool(name="ps_kv", bufs=1,
                                               space="PSUM"))
        for s in range(NSA):
            kv16 = krep = None
            if s > 0:
                kv16, krep = _kv_prefix(nc, g, s, p_kvs, ps_kv)
            yT = p_yT.tile([P, NH, SW], BF16, tag="yT")
            for h in range(NH):
                _attn_head(nc, g, s, h, kv16, krep, yT, p_e, p_rd, ps_s,
                           ps_yd)
            _out_proj(nc, g, s, yT, p_wp, p_o, ps_s)


def _kv_prefix(nc, g, s, p_kvs, ps_kv):
    """prefix K^T V and ksum per kv-head (linear branch), strictly causal."""
    kv16 = p_kvs.tile([P, NKV, P], BF16, tag="kv")
    krep = p_kvs.tile([P, NKV, P], BF16, tag="krep")
    ks = p_kvs.tile([P, NKV, 1], F32, tag="ks")
    pkv = ps_kv.tile([P, NKV, P], F32, tag="pkv")
    nck = 4 * s
    for gi in range(NKV):
        gsl = slice(gi * P, (gi + 1) * P)
        for c in range(nck):
            nc.tensor.matmul(pkv[:, gi, :], g.kTok[:, c, gi, :],
                             g.v16[:, c, gsl],
                             start=(c == 0), stop=(c == nck - 1))
        # ksum[hd'] = sum_k khat (x32): free-dim reduction of k8 prefix
        nc.vector.tensor_reduce(ks[:, gi, :], g.k8[:, gi, 0:512 * s],
                                mybir.AxisListType.X, mybir.AluOpType.add)
    nc.scalar.activation(kv16[:], pkv[:], Copy, scale=C0)
    for gi in range(NKV):
        nc.vector.tensor_scalar_mul(krep[:, gi, :],
                                    ks[:, gi, 0:1].to_broadcast([P, P]), C0)
    return kv16, krep


def _attn_head(nc, g, s, h, kv16, krep, yT, p_e, p_rd, ps_s, ps_yd):
    gi = h // 4
    gsl = slice(gi * P, (gi + 1) * P)
    lsl = slice(s * SW, (s + 1) * SW)
    kc0 = 4 * s
    npair = 2 * s
    # diagonal scores (fp8 DoubleRow over hd halves)
    psc = ps_s.tile([P, 4, SW], F32, tag="s")
    for kc in range(4):
        ksl = slice((kc0 + kc) * P, (kc0 + kc + 1) * P)
        nc.tensor.matmul(psc[:, kc, :], g.k8r[:, :, gi, ksl],
                         g.q8r[:, :, h, lsl],
                         start=True, stop=True, perf_mode=DR)
    e16 = p_e.tile([P, 4, SW], FP16, tag="e")
    nc.scalar.activation(e16[:], psc[:], Exp, scale=C0)
    nc.vector.tensor_tensor(e16[:], e16[:], g.mask_sb[:], MUL)

    pyd = ps_yd.tile([P, 2, SW], F32, tag="yd")
    py = pyd[:, 0, :]
    pden = pyd[:, 1, :]
    # numerator chain: sum_full v  +  c*q8^T K^T V  +  diag e16 @ v16
    first = True
    for c in range(npair):
        pr = slice(2 * c, 2 * c + 2)
        nc.tensor.matmul(py, g.v8[:, pr, gsl], g.ones8[:],
                         start=first, stop=False, perf_mode=DR)
        first = False
    if s > 0:
        nc.tensor.matmul(py, kv16[:, gi, :], g.q8[:, h, lsl],
                         start=first, stop=False)
        first = False
    if DR16_DIAG:
        for c in range(2):
            nc.tensor.matmul(py, g.v16[:, kc0 + 2 * c:kc0 + 2 * c + 2, gsl],
                             e16[:, 2 * c:2 * c + 2, :],
                             start=first, stop=(c == 1), perf_mode=DR)
            first = False
    else:
        for kc in range(4):
            nc.tensor.matmul(py, g.v16[:, kc0 + kc, gsl], e16[:, kc, :],
                             start=first, stop=(kc == 3))
            first = False
    # denominator chain
    first = True
    if s > 0:
        nc.tensor.matmul(pden, krep[:, gi, :], g.q8[:, h, lsl],
                         start=True, stop=False)
        first = False
    if DR16_DIAG:
        for c in range(2):
            nc.tensor.matmul(pden,
                             g.ones_h[:, None, :].to_broadcast([P, 2, P]),
                             e16[:, 2 * c:2 * c + 2, :],
                             start=first, stop=(c == 1), perf_mode=DR)
            first = False
    else:
        for kc in range(4):
            nc.tensor.matmul(pden, g.ones_h[:], e16[:, kc, :],
                             start=first, stop=(kc == 3))
            first = False
    # rden = 1 / (pden + 512*s); yT = py * rden
    rden = p_rd.tile([P, SW], F32, tag="rd")
    if s > 0:
        tden = p_rd.tile([P, SW], F32, tag="td")
        nc.vector.tensor_scalar_add(tden[:], pden, float(512 * s))
        nc.vector.reciprocal_approx_fast(rden[:], tden[:])
    else:
        nc.vector.reciprocal_approx_fast(rden[:], pden)
    nc.vector.tensor_tensor(yT[:, h, :], py, rden[:], MUL)


def _out_proj(nc, g, s, yT, p_wp, p_o, ps_s):
    lsl = slice(s * SW, (s + 1) * SW)
    for og in range(8):
        wp_t = p_wp.tile([P, NH, 2 * P], BF16, tag="wpt")
        nc.sync.dma_start(
            wp_t[:],
            g.wp[:, og * 2 * P:(og + 1) * 2 * P].rearrange(
                "(hh p) f -> p hh f", p=P))
        o_sb = p_o.tile([P, 2, SW], F32, tag="o")
        for j in range(2):
            po = ps_s.tile([P, SW], F32, tag="s")
            if DR16_OUT:
                for c in range(NH // 2):
                    nc.tensor.matmul(
                        po[:], wp_t[:, 2 * c:2 * c + 2, j * P:(j + 1) * P],
                        yT[:, 2 * c:2 * c + 2, :],
                        start=(c == 0), stop=(c == 7), perf_mode=DR)
            else:
                for hh in range(NH):
                    nc.tensor.matmul(po[:], wp_t[:, hh, j * P:(j + 1) * P],
                                     yT[:, hh, :],
                                     start=(hh == 0), stop=(hh == NH - 1))
            nc.vector.tensor_copy(o_sb[:, j, :], po[:])
        nc.sync.dma_start(
            g.out.rearrange("(og j p) q -> p og j q", p=P, j=2)[:, og, :,
                                                                lsl],
            o_sb[:])


def build():
    nc = bacc.Bacc("TRN2", target_bir_lowering=False, debug=False,
                   num_devices=N_CORES)
    g = SimpleNamespace()
    g.xb = nc.declare_dram_parameter("xb", [T, C], BF16, isOutput=False)
    g.wq = nc.declare_dram_parameter("wq", [C, C], FP8, isOutput=False)
    g.wk = nc.declare_dram_parameter("wk", [C, KV], FP8, isOutput=False)
    g.wv = nc.declare_dram_parameter("wv", [C, KV], FP8, isOutput=False)
    g.wp = nc.declare_dram_parameter("wp", [C, C], BF16, isOutput=False)
    masks = nc.declare_dram_parameter("masks", [4, P, SW], FP16,
                                      isOutput=False)
    ident_in = nc.declare_dram_parameter("ident", [P, P], BF16, isOutput=False)
    onesb_in = nc.declare_dram_parameter("onesb", [P, P], BF16, isOutput=False)
    onesh_in = nc.declare_dram_parameter("onesh", [P, P], FP16, isOutput=False)
    ones8_in = nc.declare_dram_parameter("ones8", [P, 2, SW], FP8,
                                         isOutput=False)
    g.out = nc.declare_dram_parameter("out", [C, LQ], F32, isOutput=True)

    with tile.TileContext(nc) as tc, ExitStack() as ctx:
        cst = ctx.enter_context(tc.tile_pool(name="cst", bufs=1))
        p_kT = ctx.enter_context(tc.tile_pool(name="kTb", bufs=1))
        p_k8 = ctx.enter_context(tc.tile_pool(name="k8p", bufs=1))
        p_ktok = ctx.enter_context(tc.tile_pool(name="ktok", bufs=1))
        p_v16 = ctx.enter_context(tc.tile_pool(name="v16p", bufs=1))
        p_v8 = ctx.enter_context(tc.tile_pool(name="v8p", bufs=1))
        p_q8 = ctx.enter_context(tc.tile_pool(name="q8p", bufs=1))
        p_xqh = ctx.enter_context(tc.tile_pool(name="xqh", bufs=1))
        p_rep = ctx.enter_context(tc.tile_pool(name="rep", bufs=1))

        g.ident = cst.tile([P, P], BF16)
        nc.sync.dma_start(g.ident[:], ident_in[:])
        g.ones_bf = cst.tile([P, P], BF16)
        nc.sync.dma_start(g.ones_bf[:], onesb_in[:])
        g.ones_h = cst.tile([P, P], FP16)
        nc.sync.dma_start(g.ones_h[:], onesh_in[:])
        g.ones8 = cst.tile([P, 2, SW], FP8)
        nc.sync.dma_start(g.ones8[:], ones8_in[:])
        g.mask_sb = cst.tile([P, 4, SW], FP16)
        nc.sync.dma_start(g.mask_sb[:], masks.rearrange("j p f -> p j f"))

        g.kT_sb = p_kT.tile([P, NM_K, T], BF16)       # 16 KB/part (32*khat)
        g.k8 = p_k8.tile([P, NM_K, T], FP8)           # 8 KB
        g.kTok = p_ktok.tile([P, NCC, NM_K, P], FP8)  # 8 KB token-major
        g.v16 = p_v16.tile([P, NCC, KV], FP16)        # 16 KB (32*v)
        g.v8 = p_v8.tile([P, NCC, KV], FP8)           # 8 KB
        g.q8 = p_q8.tile([P, NM_Q, LQ], FP8)          # 16 KB (32*qhat)
        g.xqh = p_xqh.tile([P, NCC, LQ], FP8)         # 16 KB query rows^T
        g.q8r = p_rep.tile([64, 2, NM_Q, LQ], FP8)    # 32 KB on parts 0..63
        g.k8r = p_rep.tile([64, 2, NM_K, T], FP8)     # 16 KB on parts 0..63

        _phase_k(nc, tc, g)
        _phase_q_body(nc, tc, g)

        # repack q8/k8 to [64, 2(hd-half), ...] for DoubleRow scores
        nc.sync.dma_start(g.q8r[:, 0], g.q8[0:64])
        nc.sync.dma_start(g.q8r[:, 1], g.q8[64:128])
        nc.sync.dma_start(g.k8r[:, 0], g.k8[0:64])
        nc.sync.dma_start(g.k8r[:, 1], g.k8[64:128])

        _phase_a(nc, tc, g)

    nc.compile()
    return nc


_NC = None


def _get_nc():
    global _NC
    if _NC is None:
        _NC = build()
    return _NC


def _make_masks(p: int) -> np.ndarray:
    j = np.arange(4)[:, None, None]
    k = np.arange(P)[None, :, None]
    q = np.arange(SW)[None, None, :]
    kk = k if p == 0 else (k ^ 1)
    valid = (2 * q + p) >= (128 * j + kk)
    return valid.astype(np.float16)


def _fp8(a: np.ndarray) -> np.ndarray:
    return np.clip(a, -230.0, 230.0).astype(ml_dtypes.float8_e4m3)


def kernel(x, Wq, Wkv, Wproj):
    x = np.asarray(x, dtype=np.float32)
    Wq = np.asarray(Wq, dtype=np.float32)
    Wkv = np.asarray(Wkv, dtype=np.float32)
    Wproj = np.asarray(Wproj, dtype=np.float32)

    wq8 = _fp8(WS * Wq)
    wk8 = _fp8(WS * np.ascontiguousarray(Wkv[:, :KV]))
    wv8 = _fp8(WS * np.ascontiguousarray(Wkv[:, KV:]))
    wp16 = (Wproj / WS).astype(ml_dtypes.bfloat16)
    ident = np.eye(P, dtype=ml_dtypes.bfloat16)
    onesb = np.ones((P, P), dtype=ml_dtypes.bfloat16)
    onesh = np.ones((P, P), dtype=np.float16)
    ones8 = np.ones((P, 2, SW), dtype=ml_dtypes.float8_e4m3)
    masks_by_p = [_make_masks(0), _make_masks(1)]

    in_maps = []
    for c in range(N_CORES):
        b, p = c // 2, c % 2
        if p == 0:
            xb_c = x[b].astype(ml_dtypes.bfloat16)
        else:
            xb_c = (x[b].reshape(T // 2, 2, C)[:, ::-1, :]
                    .reshape(T, C).astype(ml_dtypes.bfloat16))
        in_maps.append({
            "xb": np.ascontiguousarray(xb_c),
            "wq": wq8, "wk": wk8, "wv": wv8, "wp": wp16,
            "masks": masks_by_p[p],
            "ident": ident, "onesb": onesb, "onesh": onesh, "ones8": ones8,
        })

    nc = _get_nc()
    res = run_bass_kernel_spmd(nc, in_maps, list(range(N_CORES)),
                               trace=False)

    result = np.empty((B, T, C), dtype=np.float32)
    for c in range(N_CORES):
        b, p = c // 2, c % 2
        result[b, p::2, :] = res.results[c]["out"].T
    return result
